# revision 1
# baseline (speedup 1.0000x reference)
"""Trainium2 Bass kernel for an AST-generation head (batch-data-parallel, 8 cores).

Shapes (hardcoded): hidden_states [256, 128, 512] fp32.
Outputs: (num_nodes_logits [256,30], node_exist [256,30], node_types [256,30,30],
          node_values [256,30], adjacency [256,30,30]).
"""

import numpy as np

import concourse.bass as bass
import concourse.tile as tile
from concourse import bacc, mybir
from concourse.bass_utils import run_bass_kernel_spmd

B, S, H, N, NT = 256, 128, 512, 30, 30
HH = H // 2  # 256
NCORES = 8
BPC = B // NCORES   # 32 batches per core
GB = 8              # batches per pipeline group
NG = BPC // GB      # 4 groups
HC = H // 128       # 4 H-chunks
KC = HH // 128      # 2 k-chunks for the adjacency bottleneck

FP32 = mybir.dt.float32
AF = mybir.ActivationFunctionType

_CACHE = {}
LAST_RESULT = None  # BassKernelResults of the most recent run (for profiling)


def _build_nc():
    nc = bacc.Bacc("TRN2", target_bir_lowering=False, debug=False, num_devices=NCORES)

    hid = nc.dram_tensor("hidden", [BPC, S, H], FP32, kind="ExternalInput")
    w_adj1T = nc.dram_tensor("w_adj1T", [H, 2 * HH], FP32, kind="ExternalInput")
    w_stackT = nc.dram_tensor("w_stackT", [H, 62], FP32, kind="ExternalInput")
    w2c = nc.dram_tensor("w2c", [HH, 1], FP32, kind="ExternalInput")
    badj1 = nc.dram_tensor("badj1", [HH, 1], FP32, kind="ExternalInput")
    bstack = nc.dram_tensor("bstack", [62, 1], FP32, kind="ExternalInput")
    bnum = nc.dram_tensor("bnum", [NT, 1], FP32, kind="ExternalInput")
    ident = nc.dram_tensor("ident30", [N, N], FP32, kind="ExternalInput")

    exist_out = nc.dram_tensor("exist_out", [1, BPC * N], FP32, kind="ExternalOutput")
    val_out = nc.dram_tensor("val_out", [1, BPC * N], FP32, kind="ExternalOutput")
    types_out = nc.dram_tensor("types_out", [NT, BPC * N], FP32, kind="ExternalOutput")
    nn_out = nc.dram_tensor("nn_out", [NT, BPC], FP32, kind="ExternalOutput")
    adj_out = nc.dram_tensor("adj_out", [BPC, N * N], FP32, kind="ExternalOutput")

    with tile.TileContext(nc) as tc:
        with (
            tc.tile_pool(name="consts", bufs=1) as consts,
            tc.tile_pool(name="hidp", bufs=2) as hidp,
            tc.tile_pool(name="nodep", bufs=2 * HC) as nodep,
            tc.tile_pool(name="abp", bufs=2) as abp,
            tc.tile_pool(name="pairp", bufs=3) as pairp,
            tc.tile_pool(name="outp", bufs=2) as outp,
            tc.tile_pool(name="grp_ps", bufs=1, space="PSUM") as grp_ps,
            tc.tile_pool(name="trans_ps", bufs=2, space="PSUM") as trans_ps,
            tc.tile_pool(name="ab_ps", bufs=2, space="PSUM") as ab_ps,
            tc.tile_pool(name="head_ps", bufs=1, space="PSUM") as head_ps,
            tc.tile_pool(name="adj_ps", bufs=1, space="PSUM") as adj_ps,
        ):
            # ---- constants in SBUF ----
            w1_sb = consts.tile([128, HC, 2 * HH], FP32, tag="w1")
            nc.sync.dma_start(
                out=w1_sb, in_=w_adj1T.ap().rearrange("(c p) m -> p c m", p=128)
            )
            ws_sb = consts.tile([128, HC, 62], FP32, tag="ws")
            nc.sync.dma_start(
                out=ws_sb, in_=w_stackT.ap().rearrange("(c p) m -> p c m", p=128)
            )
            w2_sb = consts.tile([128, KC], FP32, tag="w2")
            nc.sync.dma_start(
                out=w2_sb, in_=w2c.ap().rearrange("(c p) m -> p (c m)", p=128)
            )
            badj1_sb = consts.tile([128, KC], FP32, tag="badj1")
            nc.sync.dma_start(
                out=badj1_sb, in_=badj1.ap().rearrange("(c p) m -> p (c m)", p=128)
            )
            bstack_sb = consts.tile([62, 1], FP32, tag="bstack")
            nc.sync.dma_start(out=bstack_sb, in_=bstack.ap())
            bnum_sb = consts.tile([NT, 1], FP32, tag="bnum")
            nc.sync.dma_start(out=bnum_sb, in_=bnum.ap())
            id_sb = consts.tile([N, N], FP32, tag="ident")
            nc.sync.dma_start(out=id_sb, in_=ident.ap())
            ones_sb = consts.tile([128, 1], FP32, tag="ones")
            nc.vector.memset(ones_sb, 1.0)

            head_sb = consts.tile([62, BPC * N], FP32, tag="head_sb")
            gr_psum = grp_ps.tile([128, HC * BPC], FP32, tag="gr")

            hid_ap = hid.ap()  # [BPC, S, H]

            for g in range(NG):
                b0 = g * GB
                # ---- load hidden tiles for this group: [S=128, GB, H] ----
                ht = hidp.tile([128, GB, H], FP32, tag="hid")
                nc.sync.dma_start(
                    out=ht, in_=hid_ap[b0 : b0 + GB].transpose([1, 0, 2])
                )

                # ---- column sums over S (for the mean) ----
                for bi in range(GB):
                    for hc in range(HC):
                        nc.tensor.matmul(
                            gr_psum[:, hc * BPC + b0 + bi : hc * BPC + b0 + bi + 1],
                            lhsT=ht[:, bi, hc * 128 : (hc + 1) * 128],
                            rhs=ones_sb,
                            start=True,
                            stop=True,
                        )

                # ---- transpose node rows -> nodeT [128(H-chunk), GB*N] ----
                node_sb = []
                for hc in range(HC):
                    tp = trans_ps.tile([128, GB * N], FP32, tag="trans")
                    for bi in range(GB):
                        nc.tensor.transpose(
                            tp[:, bi * N : (bi + 1) * N],
                            ht[0:N, bi, hc * 128 : (hc + 1) * 128],
                            id_sb,
                        )
                    nsb = nodep.tile([128, GB * N], FP32, tag="node")
                    nc.scalar.copy(nsb, tp)
                    node_sb.append(nsb)

                # ---- ai/aj matmuls: 4 m-chunks of [128, GB*N] ----
                ab_sb = []
                for mc in range(4):
                    ap_ps = ab_ps.tile([128, GB * N], FP32, tag="ab")
                    for hc in range(HC):
                        nc.tensor.matmul(
                            ap_ps,
                            lhsT=w1_sb[:, hc, mc * 128 : (mc + 1) * 128],
                            rhs=node_sb[hc],
                            start=(hc == 0),
                            stop=(hc == HC - 1),
                        )
                    sb = abp.tile([128, GB * N], FP32, tag=f"ab{mc}")
                    if mc < KC:  # ai chunks: fold in b_adj1
                        nc.scalar.activation(
                            sb, ap_ps, AF.Identity,
                            bias=badj1_sb[:, mc : mc + 1], scale=1.0,
                        )
                    else:
                        nc.scalar.copy(sb, ap_ps)
                    ab_sb.append(sb)

                # ---- head matmuls ----
                hp = head_ps.tile([62, GB * N], FP32, tag="head")
                for hc in range(HC):
                    nc.tensor.matmul(
                        hp,
                        lhsT=ws_sb[:, hc, :],
                        rhs=node_sb[hc],
                        start=(hc == 0),
                        stop=(hc == HC - 1),
                    )
                nc.scalar.activation(
                    head_sb[:, b0 * N : (b0 + GB) * N], hp, AF.Identity,
                    bias=bstack_sb, scale=1.0,
                )

                # ---- pairwise adjacency ----
                for ag in range(GB // 4):  # 2 sub-groups of 4 batches
                    adj_psum = adj_ps.tile([128, 1024], FP32, tag="adj")
                    for jj in range(4):
                        bi = ag * 4 + jj
                        for c in range(KC):
                            ai_sl = ab_sb[c][:, bi * N : (bi + 1) * N]
                            aj_sl = ab_sb[KC + c][:, bi * N : (bi + 1) * N]
                            pr = pairp.tile([128, N, N], FP32, tag="pair_sum")
                            nc.vector.tensor_add(
                                pr,
                                ai_sl.unsqueeze(2).broadcast_to([128, N, N]),
                                aj_sl.unsqueeze(1).broadcast_to([128, N, N]),
                            )
                            pf = pairp.tile([128, N, N], FP32, tag="pair_relu")
                            nc.scalar.activation(pf, pr, AF.Relu)
                            pflat = pf.rearrange("p a b -> p (a b)")
                            for nh in range(2):
                                nc.tensor.matmul(
                                    adj_psum[
                                        32 * jj : 32 * jj + 1,
                                        nh * 512 : nh * 512 + 450,
                                    ],
                                    lhsT=w2_sb[:, c : c + 1],
                                    rhs=pflat[:, nh * 450 : (nh + 1) * 450],
                                    start=(c == 0),
                                    stop=(c == KC - 1),
                                    tile_position=(0, 32 * jj),
                                )
                    # compact copy PSUM -> SBUF (4 batches on partitions 0/32/64/96)
                    adj_sb = outp.tile([128, 2, 450], FP32, tag="adj_sb")
                    src = bass.AP(
                        tensor=adj_psum.tensor,
                        offset=adj_psum.offset,
                        ap=[adj_psum.ap[0], [512, 2], [1, 450]],
                    )
                    nc.vector.tensor_copy(adj_sb, src)
                    # DMA the 4 batch rows out
                    gbase = b0 + ag * 4
                    dst = bass.AP(
                        tensor=adj_out,
                        offset=gbase * N * N,
                        ap=[[N * N, 4], [450, 2], [1, 450]],
                    )
                    src2 = bass.AP(
                        tensor=adj_sb.tensor,
                        offset=adj_sb.offset,
                        ap=[[adj_sb.ap[0][0] * 32, 4], [450, 2], [1, 450]],
                    )
                    nc.sync.dma_start(out=dst, in_=src2)

            # ---- num_nodes from global mean ----
            gr_sb = consts.tile([128, HC * BPC], FP32, tag="gr_sb")
            nc.vector.tensor_copy(gr_sb, gr_psum)
            nn_ps = trans_ps.tile([NT, BPC], FP32, tag="trans")
            for hc in range(HC):
                nc.tensor.matmul(
                    nn_ps,
                    lhsT=ws_sb[:, hc, 32:62],
                    rhs=gr_sb[:, hc * BPC : (hc + 1) * BPC],
                    start=(hc == 0),
                    stop=(hc == HC - 1),
                )
            nn_sb = consts.tile([NT, BPC], FP32, tag="nn_sb")
            nc.scalar.activation(nn_sb, nn_ps, AF.Identity, bias=bnum_sb, scale=1.0)
            nc.sync.dma_start(out=nn_out.ap(), in_=nn_sb)

            # ---- head output DMAs ----
            nc.sync.dma_start(out=exist_out.ap(), in_=head_sb[0:1, :])
            nc.sync.dma_start(out=val_out.ap(), in_=head_sb[1:2, :])
            nc.sync.dma_start(out=types_out.ap(), in_=head_sb[2:32, :])

    nc.compile()
    return nc


def _get_nc():
    if "nc" not in _CACHE:
        _CACHE["nc"] = _build_nc()
    return _CACHE["nc"]


def kernel(
    hidden_states, w_exist, b_exist, w_type, b_type, w_val, b_val,
    w_adj1, b_adj1, w_adj2, b_adj2, w_num, b_num,
):
    global LAST_RESULT
    hidden_states = np.ascontiguousarray(np.asarray(hidden_states, np.float32))
    w_adj1 = np.asarray(w_adj1, np.float32)

    # host-side weight packing (tiny, batch-independent)
    w1a, w1b = w_adj1[:, :H], w_adj1[:, H:]            # [HH, H] each
    w_adj1T = np.ascontiguousarray(np.concatenate([w1a, w1b], 0).T)  # [H, 2*HH]
    w_stackT = np.ascontiguousarray(
        np.concatenate(
            [np.asarray(w_exist, np.float32), np.asarray(w_val, np.float32),
             np.asarray(w_type, np.float32), np.asarray(w_num, np.float32) / S],
            axis=0,
        ).T
    )  # [H, 62]
    w2c = np.ascontiguousarray(np.asarray(w_adj2, np.float32).T)      # [HH, 1]
    badj1 = np.ascontiguousarray(np.asarray(b_adj1, np.float32)[:, None])
    bstack = np.ascontiguousarray(
        np.concatenate(
            [np.asarray(b_exist, np.float32), np.asarray(b_val, np.float32),
             np.asarray(b_type, np.float32), np.asarray(b_num, np.float32)]
        )[:, None]
    )  # [62, 1]
    bnum = np.ascontiguousarray(np.asarray(b_num, np.float32)[:, None])
    ident = np.eye(N, dtype=np.float32)

    shared = {
        "w_adj1T": w_adj1T, "w_stackT": w_stackT, "w2c": w2c,
        "badj1": badj1, "bstack": bstack, "bnum": bnum, "ident30": ident,
    }
    in_maps = [
        {"hidden": hidden_states[c * BPC : (c + 1) * BPC], **shared}
        for c in range(NCORES)
    ]

    nc = _get_nc()
    res = run_bass_kernel_spmd(nc, in_maps, core_ids=list(range(NCORES)))
    LAST_RESULT = res

    num_nodes = np.empty((B, NT), np.float32)
    exist = np.empty((B, N), np.float32)
    values = np.empty((B, N), np.float32)
    types = np.empty((B, N, NT), np.float32)
    adjacency = np.empty((B, N, N), np.float32)
    for c in range(NCORES):
        r = res.results[c]
        sl = slice(c * BPC, (c + 1) * BPC)
        num_nodes[sl] = r["nn_out"].T
        exist[sl] = r["exist_out"].reshape(BPC, N)
        values[sl] = r["val_out"].reshape(BPC, N)
        types[sl] = r["types_out"].reshape(NT, BPC, N).transpose(1, 2, 0)
        adjacency[sl] = r["adj_out"].reshape(BPC, N, N)

    adjacency += np.asarray(b_adj2, np.float32)[0]
    idx = np.arange(N)
    adjacency[:, idx, idx] = 0.0
    return (num_nodes, exist, types, values, adjacency)


# revision 4
# speedup vs baseline: 1.0942x; 1.0942x over previous
"""Trainium2 Bass kernel for an AST-generation head (batch-data-parallel, 8 cores).

Shapes (hardcoded): hidden_states [256, 128, 512] fp32.
Outputs: (num_nodes_logits [256,30], node_exist [256,30], node_types [256,30,30],
          node_values [256,30], adjacency [256,30,30]).

Per-core plan (32 batches):
 - one DMA per 8-batch group loads [S=128(part), b, H] tiles
 - a single PE transpose-matmul per (batch, H-chunk) against a constant
   [identity30 | ones128] selector yields nodeT AND the S-column-sums (mean)
 - heads contract H on partitions in fp32; ai/aj in fp16
 - pairwise relu(ai + aj + b) built in a diagonal layout so both DVE add
   operands are stride-1 (fp16 2x mode); relu in-place at 4x mode
 - adjacency = w2 @ pair via fp16 matmuls, 4 batches packed per PSUM bank
   via column tile_position; host unpermutes the diagonal ordering
"""

import numpy as np

import concourse.bass as bass
import concourse.tile as tile
from concourse import bacc, mybir
from concourse.bass_utils import run_bass_kernel_spmd

B, S, H, N, NT = 256, 128, 512, 30, 30
HH = H // 2  # 256
NCORES = 8
BPC = B // NCORES   # 32 batches per core
GB = 8              # batches per pipeline group
NG = BPC // GB      # 4 groups
HC = H // 128       # 4 H-chunks
KC = HH // 128      # 2 k-chunks for the adjacency bottleneck

FP32 = mybir.dt.float32
FP16 = mybir.dt.float16
AF = mybir.ActivationFunctionType

_CACHE = {}
LAST_RESULT = None  # BassKernelResults of the most recent run (for profiling)


def _build_nc():
    nc = bacc.Bacc("TRN2", target_bir_lowering=False, debug=False, num_devices=NCORES)

    hid = nc.dram_tensor("hidden", [BPC, S, H], FP32, kind="ExternalInput")
    w_adj1T = nc.dram_tensor("w_adj1T", [H, 2 * HH], FP16, kind="ExternalInput")
    w_stackT = nc.dram_tensor("w_stackT", [H, 62], FP32, kind="ExternalInput")
    w2c = nc.dram_tensor("w2c", [HH, 1], FP16, kind="ExternalInput")
    badj1 = nc.dram_tensor("badj1", [HH, 1], FP32, kind="ExternalInput")
    bstack = nc.dram_tensor("bstack", [62, 1], FP32, kind="ExternalInput")
    bnum = nc.dram_tensor("bnum", [NT, 1], FP32, kind="ExternalInput")
    ident = nc.dram_tensor("ident30", [N, N], FP32, kind="ExternalInput")

    exist_out = nc.dram_tensor("exist_out", [1, BPC * N], FP32, kind="ExternalOutput")
    val_out = nc.dram_tensor("val_out", [1, BPC * N], FP32, kind="ExternalOutput")
    types_out = nc.dram_tensor("types_out", [NT, BPC * N], FP32, kind="ExternalOutput")
    nn_out = nc.dram_tensor("nn_out", [NT, BPC], FP32, kind="ExternalOutput")
    adj_out = nc.dram_tensor("adj_out", [BPC, N * N], FP32, kind="ExternalOutput")

    with tile.TileContext(nc) as tc:
        with (
            tc.tile_pool(name="consts", bufs=1) as consts,
            tc.tile_pool(name="hidp", bufs=2) as hidp,
            tc.tile_pool(name="nodep", bufs=2 * HC) as nodep,
            tc.tile_pool(name="abp", bufs=2) as abp,
            tc.tile_pool(name="pairp", bufs=2) as pairp,
            tc.tile_pool(name="outp", bufs=2) as outp,
            tc.tile_pool(name="trans_ps", bufs=2, space="PSUM") as trans_ps,
            tc.tile_pool(name="ab_ps", bufs=2, space="PSUM") as ab_ps,
            tc.tile_pool(name="head_ps", bufs=1, space="PSUM") as head_ps,
            tc.tile_pool(name="grp_ps", bufs=1, space="PSUM") as grp_ps,
            tc.tile_pool(name="adj_ps", bufs=1, space="PSUM") as adj_ps,
        ):
            # ---- constants in SBUF ----
            w1_sb = consts.tile([128, HC, 2 * HH], FP16, tag="w1")
            nc.sync.dma_start(
                out=w1_sb, in_=w_adj1T.ap().rearrange("(c p) m -> p c m", p=128)
            )
            ws_sb = consts.tile([128, HC, 62], FP32, tag="ws")
            nc.sync.dma_start(
                out=ws_sb, in_=w_stackT.ap().rearrange("(c p) m -> p c m", p=128)
            )
            w2_sb = consts.tile([128, KC], FP16, tag="w2")
            nc.sync.dma_start(
                out=w2_sb, in_=w2c.ap().rearrange("(c p) m -> p (c m)", p=128)
            )
            badj1_sb = consts.tile([128, KC], FP32, tag="badj1")
            nc.sync.dma_start(
                out=badj1_sb, in_=badj1.ap().rearrange("(c p) m -> p (c m)", p=128)
            )
            bstack_sb = consts.tile([62, 1], FP32, tag="bstack")
            nc.sync.dma_start(out=bstack_sb, in_=bstack.ap())
            bnum_sb = consts.tile([NT, 1], FP32, tag="bnum")
            nc.sync.dma_start(out=bnum_sb, in_=bnum.ap())
            id_sb = consts.tile([N, N], FP32, tag="ident")
            nc.sync.dma_start(out=id_sb, in_=ident.ap())
            ones_sb = consts.tile([128, 1], FP32, tag="ones")
            nc.vector.memset(ones_sb, 1.0)

            head_sb = consts.tile([62, BPC * N], FP32, tag="head_sb")
            gr_sb = consts.tile([128, HC, BPC], FP32, tag="gr_sb")
            gr_psum = grp_ps.tile([128, HC * BPC], FP32, tag="gr")

            hid_ap = hid.ap()  # [BPC, S, H]

            for g in range(NG):
                b0 = g * GB
                with nc.named_scope(f"load{g}"):
                    ht = hidp.tile([128, GB, H], FP32, tag="hid")
                    nc.sync.dma_start(
                        out=ht, in_=hid_ap[b0 : b0 + GB].transpose([1, 0, 2])
                    )

                # ---- column sums over S (for the mean) ----
                with nc.named_scope("mean_mm"):
                    for bi in range(GB):
                        for hc in range(HC):
                            col = hc * BPC + b0 + bi
                            nc.tensor.matmul(
                                gr_psum[:, col : col + 1],
                                lhsT=ht[:, bi, hc * 128 : (hc + 1) * 128],
                                rhs=ones_sb,
                                start=True,
                                stop=True,
                            )

                # ---- transpose node rows -> nodeT [128(H-chunk), GB, N] ----
                node32, node16 = [], []
                for hc in range(HC):
                    with nc.named_scope("transp"):
                        tp = trans_ps.tile([128, GB, N], FP32, tag="trans")
                        for bi in range(GB):
                            nc.tensor.transpose(
                                tp[:, bi, :],
                                ht[0:N, bi, hc * 128 : (hc + 1) * 128],
                                id_sb,
                            )
                    with nc.named_scope("node_copy"):
                        n32 = nodep.tile([128, GB, N], FP32, tag="node32")
                        nc.scalar.copy(n32, tp)
                        n16 = nodep.tile([128, GB, N], FP16, tag="node16")
                        nc.scalar.copy(n16, tp)
                        node32.append(n32)
                        node16.append(n16)

                # ---- ai/aj matmuls (fp16) ----
                ai_all = abp.tile([128, KC, GB, 32], FP16, tag="ai")
                aj_ext = abp.tile([128, KC, GB, 64], FP16, tag="aje")
                aj_ext1 = abp.tile([128, KC, GB, 64], FP16, tag="aje1")
                for mc in range(4):
                    with nc.named_scope("aiaj_mm"):
                        ab = ab_ps.tile([128, GB, N], FP32, tag="ab")
                        for hc in range(HC):
                            nc.tensor.matmul(
                                ab.rearrange("p b d -> p (b d)"),
                                lhsT=w1_sb[:, hc, mc * 128 : (mc + 1) * 128],
                                rhs=node16[hc].rearrange("p b d -> p (b d)"),
                                start=(hc == 0),
                                stop=(hc == HC - 1),
                            )
                    with nc.named_scope("ab_copy"):
                        if mc < KC:  # ai chunk: fold in b_adj1, cast fp16
                            nc.scalar.activation(
                                ai_all[:, mc, :, 0:N], ab, AF.Identity,
                                bias=badj1_sb[:, mc : mc + 1], scale=1.0,
                            )
                        else:
                            c = mc - KC
                            nc.scalar.copy(aj_ext[:, c, :, 0:N], ab)
                            nc.scalar.copy(aj_ext[:, c, :, N : 2 * N], ab)
                            nc.scalar.copy(
                                aj_ext1[:, c, :, 0 : N - 1], ab[:, :, 1:N]
                            )
                            nc.scalar.copy(
                                aj_ext1[:, c, :, N - 1 : 2 * N - 1], ab
                            )

                # ---- heads (fp32) ----
                with nc.named_scope("heads"):
                    hp = head_ps.tile([62, GB * N], FP32, tag="head")
                    for hc in range(HC):
                        nc.tensor.matmul(
                            hp,
                            lhsT=ws_sb[:, hc, :],
                            rhs=node32[hc].rearrange("p b d -> p (b d)"),
                            start=(hc == 0),
                            stop=(hc == HC - 1),
                        )
                    nc.scalar.activation(
                        head_sb[:, b0 * N : (b0 + GB) * N], hp, AF.Identity,
                        bias=bstack_sb, scale=1.0,
                    )

                # ---- pairwise pre-activations, diagonal layout (fp16) ----
                pair = pairp.tile([128, N, KC, GB, 32], FP16, tag="pair")
                with nc.named_scope("pair_add"):
                    for o in range(N):
                        ext, off = (aj_ext, o) if o % 2 == 0 else (aj_ext1, o - 1)
                        nc.vector.tensor_add(
                            pair[:, o, :, :, 0:N],
                            ai_all[:, :, :, 0:N],
                            ext[:, :, :, off : off + N],
                        )
                with nc.named_scope("relu"):
                    pflat = pair.rearrange("p o c b d -> p (o c b d)")
                    nc.vector.tensor_scalar_max(pflat, pflat, 0.0)

                # ---- adjacency matmuls + output ----
                for ag in range(GB // 4):
                    with nc.named_scope("adj_mm"):
                        adj_psum = adj_ps.tile([128, 1024], FP32, tag="adj")
                        for jj in range(4):
                            bi = ag * 4 + jj
                            for c in range(KC):
                                for oh in range(2):
                                    rhs = pair[
                                        :, oh * 15 : (oh + 1) * 15, c, bi, 0:N
                                    ]
                                    nc.tensor.matmul(
                                        adj_psum[
                                            32 * jj : 32 * jj + 1,
                                            oh * 512 : oh * 512 + 450,
                                        ],
                                        lhsT=w2_sb[:, c : c + 1],
                                        rhs=rhs,
                                        start=(c == 0),
                                        stop=(c == KC - 1),
                                        tile_position=(0, 32 * jj),
                                    )
                    with nc.named_scope("adj_out"):
                        adj_sb = outp.tile([128, 2, 450], FP32, tag="adj_sb")
                        src = bass.AP(
                            tensor=adj_psum.tensor,
                            offset=adj_psum.offset,
                            ap=[adj_psum.ap[0], [512, 2], [1, 450]],
                        )
                        nc.scalar.copy(adj_sb, src)
                        gbase = b0 + ag * 4
                        dst = bass.AP(
                            tensor=adj_out,
                            offset=gbase * N * N,
                            ap=[[N * N, 4], [450, 2], [1, 450]],
                        )
                        src2 = bass.AP(
                            tensor=adj_sb.tensor,
                            offset=adj_sb.offset,
                            ap=[[adj_sb.ap[0][0] * 32, 4], [450, 2], [1, 450]],
                        )
                        nc.sync.dma_start(out=dst, in_=src2)

            # ---- num_nodes from global mean ----
            with nc.named_scope("nn"):
                nc.vector.tensor_copy(gr_sb.rearrange("p a b -> p (a b)"), gr_psum)
                nn_ps = trans_ps.tile([NT, BPC], FP32, tag="trans")
                for hc in range(HC):
                    nc.tensor.matmul(
                        nn_ps,
                        lhsT=ws_sb[:, hc, 32:62],
                        rhs=gr_sb[:, hc, :],
                        start=(hc == 0),
                        stop=(hc == HC - 1),
                    )
                nn_sb = consts.tile([NT, BPC], FP32, tag="nn_sb")
                nc.scalar.activation(nn_sb, nn_ps, AF.Identity, bias=bnum_sb, scale=1.0)
                nc.sync.dma_start(out=nn_out.ap(), in_=nn_sb)

            # ---- head output DMAs ----
            nc.sync.dma_start(out=exist_out.ap(), in_=head_sb[0:1, :])
            nc.sync.dma_start(out=val_out.ap(), in_=head_sb[1:2, :])
            nc.sync.dma_start(out=types_out.ap(), in_=head_sb[2:32, :])

    nc.compile()
    return nc


def _get_nc():
    if "nc" not in _CACHE:
        _CACHE["nc"] = _build_nc()
    return _CACHE["nc"]


# host-side unpermute of the diagonal pair ordering:
# psum col p (0..899) -> o = (p//450)*15 + (p%450)//30, d = p%30
# element is pair (i=d, j=(d+o)%30)
_P = np.arange(N * N)
_O = (_P // 450) * 15 + (_P % 450) // 30
_D = _P % N
_I_IDX = _D
_J_IDX = (_D + _O) % N


def kernel(
    hidden_states, w_exist, b_exist, w_type, b_type, w_val, b_val,
    w_adj1, b_adj1, w_adj2, b_adj2, w_num, b_num,
):
    global LAST_RESULT
    hidden_states = np.ascontiguousarray(np.asarray(hidden_states, np.float32))
    w_adj1 = np.asarray(w_adj1, np.float32)

    # host-side weight packing (tiny, batch-independent)
    w1a, w1b = w_adj1[:, :H], w_adj1[:, H:]            # [HH, H] each
    w_adj1T = np.ascontiguousarray(
        np.concatenate([w1a, w1b], 0).T.astype(np.float16)
    )  # [H, 2*HH] fp16
    w_stackT = np.ascontiguousarray(
        np.concatenate(
            [np.asarray(w_exist, np.float32), np.asarray(w_val, np.float32),
             np.asarray(w_type, np.float32), np.asarray(w_num, np.float32) / S],
            axis=0,
        ).T
    )  # [H, 62]
    w2c = np.ascontiguousarray(np.asarray(w_adj2, np.float32).T.astype(np.float16))
    badj1 = np.ascontiguousarray(np.asarray(b_adj1, np.float32)[:, None])
    bstack = np.ascontiguousarray(
        np.concatenate(
            [np.asarray(b_exist, np.float32), np.asarray(b_val, np.float32),
             np.asarray(b_type, np.float32), np.asarray(b_num, np.float32)]
        )[:, None]
    )  # [62, 1]
    bnum = np.ascontiguousarray(np.asarray(b_num, np.float32)[:, None])
    ident = np.eye(N, dtype=np.float32)

    shared = {
        "w_adj1T": w_adj1T, "w_stackT": w_stackT, "w2c": w2c,
        "badj1": badj1, "bstack": bstack, "bnum": bnum, "ident30": ident,
    }
    in_maps = [
        {"hidden": hidden_states[c * BPC : (c + 1) * BPC], **shared}
        for c in range(NCORES)
    ]

    nc = _get_nc()
    res = run_bass_kernel_spmd(nc, in_maps, core_ids=list(range(NCORES)))
    LAST_RESULT = res

    num_nodes = np.empty((B, NT), np.float32)
    exist = np.empty((B, N), np.float32)
    values = np.empty((B, N), np.float32)
    types = np.empty((B, N, NT), np.float32)
    adjacency = np.empty((B, N, N), np.float32)
    for c in range(NCORES):
        r = res.results[c]
        sl = slice(c * BPC, (c + 1) * BPC)
        num_nodes[sl] = r["nn_out"].T
        exist[sl] = r["exist_out"].reshape(BPC, N)
        values[sl] = r["val_out"].reshape(BPC, N)
        types[sl] = r["types_out"].reshape(NT, BPC, N).transpose(1, 2, 0)
        adjacency[sl][:, _I_IDX, _J_IDX] = r["adj_out"]

    adjacency += np.asarray(b_adj2, np.float32)[0]
    idx = np.arange(N)
    adjacency[:, idx, idx] = 0.0
    return (num_nodes, exist, types, values, adjacency)


# revision 19
# speedup vs baseline: 2.0973x; 1.9167x over previous
"""Trainium2 Bass kernel for an AST-generation head (batch-data-parallel, 8 cores).

Shapes (hardcoded): hidden_states [256, 128, 512] fp32.
Outputs: (num_nodes_logits [256,30], node_exist [256,30], node_types [256,30,30],
          node_values [256,30], adjacency [256,30,30]).

Per-core plan (32 batches, pipeline groups of [4, 12, 16] for fast rampup):
 - host pre-transposes hidden to [S, B, H] so each SBUF partition's DMA
   source run is contiguous (few SWDGE descriptors); SWDGE DMA loads
   hidden as fp16 (inline cast) into [S=128(part), b, H] tiles
 - one fp16 matmul per (batch, H-chunk) against a constant [ones | I30]
   selector yields the S-column-sums (mean) AND nodeT in a single pass,
   with fast weight load (FWL) since the stationary operand is fp16
 - ai/aj and heads contract H on partitions in fp16; num_nodes in fp32
 - pairwise relu(ai + aj + b) built in a diagonal layout, 32-padded so
   every DVE add operand is stride-1/4B-aligned (fp16 2x mode);
   relu in-place at 4x mode
 - adjacency = w2 @ pair via fp16 matmuls, 4 batches packed per PSUM bank
   via column tile_position; host unpermutes the diagonal ordering
Measured: ~100-115us HW exec (8 cores), worst rel err ~4e-4.
"""

import numpy as np

import concourse.bass as bass
import concourse.tile as tile
from concourse import bacc, mybir
from concourse.bass_utils import run_bass_kernel_spmd

B, S, H, N, NT = 256, 128, 512, 30, 30
HH = H // 2  # 256
NCORES = 8
BPC = B // NCORES   # 32 batches per core
GB = 16             # batches per pipeline group
NG = BPC // GB      # 2 groups
HC = H // 128       # 4 H-chunks
KC = HH // 128      # 2 k-chunks for the adjacency bottleneck

FP32 = mybir.dt.float32
FP16 = mybir.dt.float16
AF = mybir.ActivationFunctionType

_CACHE = {}
LAST_RESULT = None  # BassKernelResults of the most recent run (for profiling)


def _build_nc(pair_bufs=1):
    nc = bacc.Bacc("TRN2", target_bir_lowering=False, debug=False, num_devices=NCORES)

    hid = nc.dram_tensor("hidden", [S, BPC, H], FP32, kind="ExternalInput")
    w_adj1T = nc.dram_tensor("w_adj1T", [H, 2 * HH], FP16, kind="ExternalInput")
    w_stackT = nc.dram_tensor("w_stackT", [H, 62], FP16, kind="ExternalInput")
    w_numT = nc.dram_tensor("w_numT", [H, NT], FP32, kind="ExternalInput")
    w2c = nc.dram_tensor("w2c", [HH, 1], FP16, kind="ExternalInput")
    badj1 = nc.dram_tensor("badj1", [HH, 1], FP32, kind="ExternalInput")
    bstack = nc.dram_tensor("bstack", [62, 1], FP32, kind="ExternalInput")
    bnum = nc.dram_tensor("bnum", [NT, 1], FP32, kind="ExternalInput")
    sel = nc.dram_tensor("sel31", [S, N + 1], FP16, kind="ExternalInput")

    exist_out = nc.dram_tensor("exist_out", [1, BPC * N], FP32, kind="ExternalOutput")
    val_out = nc.dram_tensor("val_out", [1, BPC * N], FP32, kind="ExternalOutput")
    types_out = nc.dram_tensor("types_out", [NT, BPC * N], FP32, kind="ExternalOutput")
    nn_out = nc.dram_tensor("nn_out", [NT, BPC], FP32, kind="ExternalOutput")
    adj_out = nc.dram_tensor("adj_out", [BPC, N * N], FP32, kind="ExternalOutput")

    with tile.TileContext(nc) as tc:
        with (
            tc.tile_pool(name="consts", bufs=1) as consts,
            tc.tile_pool(name="hidp", bufs=1) as hidp,
            tc.tile_pool(name="nodep", bufs=2 * HC) as nodep,
            tc.tile_pool(name="abp", bufs=1) as abp,
            tc.tile_pool(name="pairp", bufs=pair_bufs) as pairp,
            tc.tile_pool(name="outp", bufs=2) as outp,
            tc.tile_pool(name="trans_ps", bufs=2, space="PSUM") as trans_ps,
            tc.tile_pool(name="ab_ps", bufs=2, space="PSUM") as ab_ps,
            tc.tile_pool(name="head_ps", bufs=1, space="PSUM") as head_ps,
            tc.tile_pool(name="adj_ps", bufs=1, space="PSUM") as adj_ps,
        ):
            # ---- constants in SBUF (small/critical first) ----
            sel_sb = consts.tile([S, N + 1], FP16, tag="sel")
            nc.sync.dma_start(out=sel_sb, in_=sel.ap())
            w2_sb = consts.tile([128, KC], FP16, tag="w2")
            nc.sync.dma_start(
                out=w2_sb, in_=w2c.ap().rearrange("(c p) m -> p (c m)", p=128)
            )
            badj1_sb = consts.tile([128, KC], FP32, tag="badj1")
            nc.sync.dma_start(
                out=badj1_sb, in_=badj1.ap().rearrange("(c p) m -> p (c m)", p=128)
            )
            bstack_sb = consts.tile([62, 1], FP32, tag="bstack")
            nc.sync.dma_start(out=bstack_sb, in_=bstack.ap())
            bnum_sb = consts.tile([NT, 1], FP32, tag="bnum")
            nc.sync.dma_start(out=bnum_sb, in_=bnum.ap())
            w1_sb = consts.tile([128, HC, 2 * HH], FP16, tag="w1")
            nc.gpsimd.dma_start(
                out=w1_sb, in_=w_adj1T.ap().rearrange("(c p) m -> p c m", p=128)
            )
            ws_sb = consts.tile([128, HC, 62], FP16, tag="ws")
            nc.gpsimd.dma_start(
                out=ws_sb, in_=w_stackT.ap().rearrange("(c p) m -> p c m", p=128)
            )
            wn_sb = consts.tile([128, HC, NT], FP32, tag="wn")

            head_sb = consts.tile([62, BPC * N], FP32, tag="head_sb")
            gr_sb = consts.tile([128, HC, BPC], FP32, tag="gr_sb")

            hid_ap = hid.ap()  # [S, BPC, H] (host pre-transposed: s-major)

            GROUPS = [(0, 4), (4, 12), (16, 16)]
            for g, (b0, gb) in enumerate(GROUPS):
                nsub = max(1, gb // 8)
                sz = gb // nsub
                with nc.named_scope(f"load{g}"):
                    hsub = []
                    for q in range(nsub):
                        t = hidp.tile([128, sz, H], FP16, tag=f"hid{g}_{q}")
                        nc.gpsimd.dma_start(
                            out=t,
                            in_=hid_ap[:, b0 + sz * q : b0 + sz * (q + 1), :],
                        )
                        hsub.append(t)

                # ---- fused selector matmul: col-sums + nodeT per (b, hc) ----
                node16 = []
                for hc in range(HC):
                    with nc.named_scope("selmm"):
                        tp = trans_ps.tile([128, gb, N + 1], FP32, tag="trans")
                        for bi in range(gb):
                            nc.tensor.matmul(
                                tp[:, bi, :],
                                lhsT=hsub[bi // sz][
                                    :, bi % sz, hc * 128 : (hc + 1) * 128
                                ],
                                rhs=sel_sb,
                                start=True,
                                stop=True,
                            )
                    with nc.named_scope("node_copy"):
                        n16 = nodep.tile([128, gb, N], FP16, tag="node16")
                        nc.scalar.copy(n16, tp[:, :, 1 : N + 1])
                        nc.scalar.copy(gr_sb[:, hc, b0 : b0 + gb], tp[:, :, 0])
                        node16.append(n16)

                # ---- ai/aj matmuls (fp16) ----
                ai_all = abp.tile([128, KC, gb, 32], FP16, tag=f"ai{g}")
                aj_ext = abp.tile([128, KC, gb, 64], FP16, tag=f"aje{g}")
                aj_ext1 = abp.tile([128, KC, gb, 64], FP16, tag=f"aje1{g}")
                for mc in range(4):
                    with nc.named_scope("aiaj_mm"):
                        ab = ab_ps.tile([128, gb, N], FP32, tag="ab")
                        for hc in range(HC):
                            nc.tensor.matmul(
                                ab.rearrange("p b d -> p (b d)"),
                                lhsT=w1_sb[:, hc, mc * 128 : (mc + 1) * 128],
                                rhs=node16[hc].rearrange("p b d -> p (b d)"),
                                start=(hc == 0),
                                stop=(hc == HC - 1),
                            )
                    with nc.named_scope("ab_copy"):
                        if mc < KC:  # ai chunk: fold in b_adj1, cast fp16
                            nc.scalar.activation(
                                ai_all[:, mc, :, 0:N], ab, AF.Identity,
                                bias=badj1_sb[:, mc : mc + 1], scale=1.0,
                            )
                        else:
                            c = mc - KC
                            nc.scalar.copy(aj_ext[:, c, :, 0:N], ab)
                            nc.scalar.copy(aj_ext[:, c, :, N : 2 * N], ab)
                            nc.scalar.copy(
                                aj_ext1[:, c, :, 0 : N - 1], ab[:, :, 1:N]
                            )
                            nc.scalar.copy(
                                aj_ext1[:, c, :, N - 1 : 2 * N - 1], ab
                            )

                # ---- heads (fp16 matmul, fp32 accumulate) ----
                with nc.named_scope("heads"):
                    hp = head_ps.tile([62, gb * N], FP32, tag="head")
                    for hc in range(HC):
                        nc.tensor.matmul(
                            hp,
                            lhsT=ws_sb[:, hc, :],
                            rhs=node16[hc].rearrange("p b d -> p (b d)"),
                            start=(hc == 0),
                            stop=(hc == HC - 1),
                        )
                    nc.scalar.activation(
                        head_sb[:, b0 * N : (b0 + gb) * N], hp, AF.Identity,
                        bias=bstack_sb, scale=1.0,
                    )

                # ---- pairwise pre-activations, diagonal layout (fp16) ----
                pair = pairp.tile([128, N, KC, gb, 32], FP16, tag=f"pair{g}")
                with nc.named_scope("pair_add"):
                    for o in range(N):
                        ext, off = (aj_ext, o) if o % 2 == 0 else (aj_ext1, o - 1)
                        nc.vector.tensor_add(
                            pair[:, o],
                            ai_all,
                            ext[:, :, :, off : off + 32],
                        )
                with nc.named_scope("relu"):
                    for oh in range(2):
                        ph = pair[:, oh * 15 : (oh + 1) * 15].rearrange(
                            "p o c b d -> p (o c b d)"
                        )
                        nc.vector.tensor_scalar_max(ph, ph, 0.0)

                # ---- adjacency matmuls + output ----
                for ag in range(gb // 4):
                    with nc.named_scope("adj_mm"):
                        adj_psum = adj_ps.tile([128, 1024], FP32, tag="adj")
                        for jj in range(4):
                            bi = ag * 4 + jj
                            for c in range(KC):
                                for oh in range(2):
                                    rhs = pair[
                                        :, oh * 15 : (oh + 1) * 15, c, bi, 0:N
                                    ]
                                    nc.tensor.matmul(
                                        adj_psum[
                                            32 * jj : 32 * jj + 1,
                                            oh * 512 : oh * 512 + 450,
                                        ],
                                        lhsT=w2_sb[:, c : c + 1],
                                        rhs=rhs,
                                        start=(c == 0),
                                        stop=(c == KC - 1),
                                        tile_position=(0, 32 * jj),
                                    )
                    with nc.named_scope("adj_out"):
                        adj_sb = outp.tile([128, 2, 450], FP32, tag="adj_sb")
                        nc.scalar.copy(adj_sb[:, 0], adj_psum[:, 0:450])
                        nc.scalar.copy(adj_sb[:, 1], adj_psum[:, 512:962])
                        gbase = b0 + ag * 4
                        dst = bass.AP(
                            tensor=adj_out,
                            offset=gbase * N * N,
                            ap=[[N * N, 4], [450, 2], [1, 450]],
                        )
                        src2 = bass.AP(
                            tensor=adj_sb.tensor,
                            offset=adj_sb.offset,
                            ap=[[adj_sb.ap[0][0] * 32, 4], [450, 2], [1, 450]],
                        )
                        nc.sync.dma_start(out=dst, in_=src2)

            # ---- num_nodes from global mean (fp32) ----
            with nc.named_scope("nn"):
                nc.gpsimd.dma_start(
                    out=wn_sb, in_=w_numT.ap().rearrange("(c p) m -> p c m", p=128)
                )
                nn_ps = trans_ps.tile([NT, BPC], FP32, tag="trans")
                for hc in range(HC):
                    nc.tensor.matmul(
                        nn_ps,
                        lhsT=wn_sb[:, hc, :],
                        rhs=gr_sb[:, hc, :],
                        start=(hc == 0),
                        stop=(hc == HC - 1),
                    )
                nn_sb = consts.tile([NT, BPC], FP32, tag="nn_sb")
                nc.scalar.activation(nn_sb, nn_ps, AF.Identity, bias=bnum_sb, scale=1.0)
                nc.sync.dma_start(out=nn_out.ap(), in_=nn_sb)

            # ---- head output DMAs ----
            nc.sync.dma_start(out=exist_out.ap(), in_=head_sb[0:1, :])
            nc.sync.dma_start(out=val_out.ap(), in_=head_sb[1:2, :])
            nc.sync.dma_start(out=types_out.ap(), in_=head_sb[2:32, :])

    nc.compile()
    return nc


def _get_nc():
    if "nc" not in _CACHE:
        _CACHE["nc"] = _build_nc()
    return _CACHE["nc"]


# host-side unpermute of the diagonal pair ordering:
# psum col p (0..899) -> o = (p//450)*15 + (p%450)//30, d = p%30
# element is pair (i=d, j=(d+o)%30)
_P = np.arange(N * N)
_O = (_P // 450) * 15 + (_P % 450) // 30
_D = _P % N
_I_IDX = _D
_J_IDX = (_D + _O) % N


def kernel(
    hidden_states, w_exist, b_exist, w_type, b_type, w_val, b_val,
    w_adj1, b_adj1, w_adj2, b_adj2, w_num, b_num,
):
    global LAST_RESULT
    hidden_states = np.ascontiguousarray(np.asarray(hidden_states, np.float32))
    w_adj1 = np.asarray(w_adj1, np.float32)

    # host-side weight packing (tiny, batch-independent)
    w1a, w1b = w_adj1[:, :H], w_adj1[:, H:]            # [HH, H] each
    w_adj1T = np.ascontiguousarray(
        np.concatenate([w1a, w1b], 0).T.astype(np.float16)
    )  # [H, 2*HH] fp16
    w_stackT = np.ascontiguousarray(
        np.concatenate(
            [np.asarray(w_exist, np.float32), np.asarray(w_val, np.float32),
             np.asarray(w_type, np.float32), np.asarray(w_num, np.float32) / S],
            axis=0,
        ).T.astype(np.float16)
    )  # [H, 62] fp16
    w_numT = np.ascontiguousarray((np.asarray(w_num, np.float32) / S).T)  # [H, NT]
    w2c = np.ascontiguousarray(np.asarray(w_adj2, np.float32).T.astype(np.float16))
    badj1 = np.ascontiguousarray(np.asarray(b_adj1, np.float32)[:, None])
    bstack = np.ascontiguousarray(
        np.concatenate(
            [np.asarray(b_exist, np.float32), np.asarray(b_val, np.float32),
             np.asarray(b_type, np.float32), np.asarray(b_num, np.float32)]
        )[:, None]
    )  # [62, 1]
    bnum = np.ascontiguousarray(np.asarray(b_num, np.float32)[:, None])
    sel = np.zeros((S, N + 1), np.float16)
    sel[:, 0] = 1.0
    sel[:N, 1 : N + 1] = np.eye(N, dtype=np.float16)

    shared = {
        "w_adj1T": w_adj1T, "w_stackT": w_stackT, "w_numT": w_numT, "w2c": w2c,
        "badj1": badj1, "bstack": bstack, "bnum": bnum, "sel31": sel,
    }
    # s-major per-core shards: [S, BPC, H] so each partition's DMA source run
    # is contiguous (cuts SWDGE descriptor count ~16x)
    hid_sm = np.ascontiguousarray(
        hidden_states.reshape(NCORES, BPC, S, H).transpose(0, 2, 1, 3)
    )
    in_maps = [{"hidden": hid_sm[c], **shared} for c in range(NCORES)]

    nc = _get_nc()
    res = run_bass_kernel_spmd(nc, in_maps, core_ids=list(range(NCORES)))
    LAST_RESULT = res

    num_nodes = np.empty((B, NT), np.float32)
    exist = np.empty((B, N), np.float32)
    values = np.empty((B, N), np.float32)
    types = np.empty((B, N, NT), np.float32)
    adjacency = np.empty((B, N, N), np.float32)
    for c in range(NCORES):
        r = res.results[c]
        sl = slice(c * BPC, (c + 1) * BPC)
        num_nodes[sl] = r["nn_out"].T
        exist[sl] = r["exist_out"].reshape(BPC, N)
        values[sl] = r["val_out"].reshape(BPC, N)
        types[sl] = r["types_out"].reshape(NT, BPC, N).transpose(1, 2, 0)
        adjacency[sl][:, _I_IDX, _J_IDX] = r["adj_out"]

    adjacency += np.asarray(b_adj2, np.float32)[0]
    idx = np.arange(N)
    adjacency[:, idx, idx] = 0.0
    return (num_nodes, exist, types, values, adjacency)
